# revision 1
# baseline (speedup 1.0000x reference)
"""Trainium2 Bass kernel for nn_BigramHash: out = tab[hash(t,prev)] @ w_proj.T.

Strategy (fold sharded by table rows, tokens routed to row owners):
  - The projection is folded into the table on-device:
        tab2 = tab @ w_proj.T
    sharded by table rows: core c computes rows [c*384, (c+1)*384).
  - The host routes each token to the core that owns its hashed row
    (the hash is recomputed on-device for the actual gather; the host
    copy is only the sharding function), padding each shard to a
    common capacity. Tokens are sorted by table row, which (a) gives
    the gather DRAM row-buffer locality and (b) lets early gather
    tiles depend only on the first fold chunks (range-based dep
    tracking), overlapping the gather with the fold.
  - Each core computes the bigram hash for its tokens on DVE (exact in
    fp32: all intermediates < 2^24), rebases it into its local slice,
    gathers rows with indirect DMA (128 rows x 4KB per instruction)
    and streams them to the output.
  - The host scatters per-core output rows back to token order.

Everything numeric from the reference (hash, fold matmul, gather) runs
on device; host work is sharding/layout marshalling and the routing
permutation. Output is bit-exact vs the fp32 reference.
"""

import numpy as np

import concourse.bass as bass
import concourse.tile as tile
from concourse import bacc, mybir
from concourse.bass_utils import run_bass_kernel_spmd

N_CORES = 8
B, T = 4, 8192
SZ, D = 3072, 1024
NTOK = B * T                      # 32768
SLICE = SZ // N_CORES             # 384 table rows per core
RC_LOC = SLICE // 128             # 3 fold row-chunks per core
KC = D // 128                     # 8 contraction chunks

C_T = 31337 % SZ                  # 617
C_P = 1000003 % SZ                # 1603

_CACHE = {}


def declare_io(nc, tiles):
    f32 = mybir.dt.float32
    i32 = mybir.dt.int32
    t_ap = nc.dram_tensor("t_sh", [128, tiles], i32, kind="ExternalInput").ap()
    tp_ap = nc.dram_tensor("tp_sh", [128, tiles], i32, kind="ExternalInput").ap()
    base_ap = nc.dram_tensor("base", [128, 1], f32, kind="ExternalInput").ap()
    tabT_ap = nc.dram_tensor(
        "tabT", [KC, 128, SLICE], f32, kind="ExternalInput"
    ).ap()
    wT_ap = nc.dram_tensor("w_projT", [KC, 128, D], f32, kind="ExternalInput").ap()
    out_ap = nc.dram_tensor("out_sh", [tiles * 128, D], f32, kind="ExternalOutput").ap()
    tab2_ap = nc.dram_tensor("tab2", [SLICE, D], f32).ap()
    return t_ap, tp_ap, base_ap, tabT_ap, wT_ap, out_ap, tab2_ap


def emit_body(nc, tc, io, tiles, bmax=None, do_fold=True, do_gather=True,
              gather_bufs=8, out_chunk=2, alt_rings=True):
    f32 = mybir.dt.float32
    i32 = mybir.dt.int32
    t_ap, tp_ap, base_ap, tabT_ap, wT_ap, out_ap, tab2_ap = io
    with (
        tc.tile_pool(name="weights", bufs=1) as wpool,
        tc.tile_pool(name="fold_out", bufs=3) as fpool,
        tc.tile_pool(name="psum", bufs=2, space="PSUM") as ppool,
        tc.tile_pool(name="idx", bufs=1) as ipool,
        tc.tile_pool(name="gather", bufs=gather_bufs) as gpool,
    ):
        # ---- load w_projT (4 MB) then tabT slice (1.5 MB) into SBUF ----
        # (w_projT gates every fold chunk — issue its loads first)
        wT_sb = []
        for kc in range(KC):
            wt = wpool.tile([128, D], f32, tag=f"wT{kc}")
            nc.scalar.dma_start(wt[:], wT_ap[kc])
            wT_sb.append(wt)
        tabT_sb = []
        for kc in range(KC):
            tt = wpool.tile([128, SLICE], f32, tag=f"tabT{kc}")
            nc.scalar.dma_start(tt[:], tabT_ap[kc])
            tabT_sb.append(tt)

        # ---- hash indices on DVE ----
        # No integer mod in the TRN2 DVE ISA; compute x % SZ exactly in
        # fp32 (all intermediates < 2^24): q = int(x/SZ) may be off by
        # one in either direction, two masked corrections fix it.
        def mod_sz(dst, src):
            m = ipool.tile([128, tiles], f32, tag="mod_m")
            nc.vector.tensor_scalar(
                m[:], src[:], 1.0 / SZ, None, op0=mybir.AluOpType.mult
            )
            qi = ipool.tile([128, tiles], i32, tag="mod_qi")
            nc.vector.tensor_copy(qi[:], m[:])
            qf = ipool.tile([128, tiles], f32, tag="mod_qf")
            nc.vector.tensor_copy(qf[:], qi[:])
            q3 = ipool.tile([128, tiles], f32, tag="mod_q3")
            nc.vector.tensor_scalar(
                q3[:], qf[:], float(SZ), None, op0=mybir.AluOpType.mult
            )
            nc.vector.tensor_tensor(dst[:], src[:], q3[:], op=mybir.AluOpType.subtract)
            fix = ipool.tile([128, tiles], f32, tag="mod_fix")
            nc.vector.tensor_scalar(
                fix[:], dst[:], 0.0, float(SZ),
                op0=mybir.AluOpType.is_lt, op1=mybir.AluOpType.mult,
            )
            nc.vector.tensor_tensor(dst[:], dst[:], fix[:], op=mybir.AluOpType.add)
            nc.vector.tensor_scalar(
                fix[:], dst[:], float(SZ), float(-SZ),
                op0=mybir.AluOpType.is_ge, op1=mybir.AluOpType.mult,
            )
            nc.vector.tensor_tensor(dst[:], dst[:], fix[:], op=mybir.AluOpType.add)

        t_sb = ipool.tile([128, tiles], i32)
        nc.scalar.dma_start(t_sb[:], t_ap[:])
        tp_sb = ipool.tile([128, tiles], i32)
        nc.scalar.dma_start(tp_sb[:], tp_ap[:])
        base_sb = ipool.tile([128, 1], f32)
        nc.scalar.dma_start(base_sb[:], base_ap[:])

        tf = ipool.tile([128, tiles], f32)
        nc.vector.tensor_copy(tf[:], t_sb[:])
        pf = ipool.tile([128, tiles], f32)
        nc.vector.tensor_copy(pf[:], tp_sb[:])

        tm = ipool.tile([128, tiles], f32)
        mod_sz(tm, tf)
        pm = ipool.tile([128, tiles], f32)
        mod_sz(pm, pf)

        s_sb = ipool.tile([128, tiles], f32)
        # s = (t % SZ)*C_T + (prev % SZ)*C_P  (< 2^23, exact in fp32)
        nc.vector.tensor_scalar(tm[:], tm[:], float(C_T), None,
                                op0=mybir.AluOpType.mult)
        nc.vector.tensor_scalar(pm[:], pm[:], float(C_P), None,
                                op0=mybir.AluOpType.mult)
        nc.vector.tensor_tensor(s_sb[:], tm[:], pm[:], op=mybir.AluOpType.add)
        sf = ipool.tile([128, tiles], f32)
        mod_sz(sf, s_sb)
        # rebase into the local slice and clamp (pad tokens may fall
        # outside this core's slice; their rows are discarded by the host)
        nc.vector.tensor_tensor(sf[:], sf[:],
                                base_sb[:, 0:1].to_broadcast([128, tiles]),
                                op=mybir.AluOpType.subtract)
        nc.vector.tensor_scalar(sf[:], sf[:], 0.0, float(SLICE - 1),
                                op0=mybir.AluOpType.max, op1=mybir.AluOpType.min)
        idx_sb = ipool.tile([128, tiles], i32)
        nc.vector.tensor_copy(idx_sb[:], sf[:])

        # ---- fold: tab2[c*SLICE:(c+1)*SLICE] = tab[rows] @ w_proj.T ----
        for rc in range(RC_LOC if do_fold else 0):
            ps = ppool.tile([128, D], f32)
            for kc in range(KC):
                lhsT = tabT_sb[kc][:, rc * 128 : (rc + 1) * 128]
                nc.tensor.matmul(
                    ps[:, 0:512], lhsT, wT_sb[kc][:, 0:512],
                    start=(kc == 0), stop=(kc == KC - 1),
                )
                nc.tensor.matmul(
                    ps[:, 512:1024], lhsT, wT_sb[kc][:, 512:1024],
                    start=(kc == 0), stop=(kc == KC - 1),
                )
            fo = fpool.tile([128, D], f32)
            nc.vector.tensor_copy(fo[:], ps[:])
            nc.sync.dma_start(tab2_ap[rc * 128 : (rc + 1) * 128, :], fo[:])

        # ---- gather + write out ----
        # bmax[j]: highest 128-row fold chunk tile j touches (tokens are
        # sorted by index, so early tiles only need early fold chunks —
        # range-based dep tracking lets those gathers overlap later fold
        # chunks). None -> conservative full span.
        for j0 in range(0, tiles if do_gather else 0, out_chunk):
            k = min(out_chunk, tiles - j0)
            g = gpool.tile([128, k * D], f32)
            for jj in range(k):
                j = j0 + jj
                span = SLICE if bmax is None else 128 * (bmax[j] + 1)
                nc.gpsimd.indirect_dma_start(
                    out=g[:, jj * D : (jj + 1) * D],
                    out_offset=None,
                    in_=tab2_ap[0:span, :],
                    in_offset=bass.IndirectOffsetOnAxis(
                        ap=idx_sb[:, j : j + 1], axis=0
                    ),
                )
            out_eng = nc.scalar if (alt_rings and (j0 // out_chunk) % 2) else nc.sync
            out_eng.dma_start(
                out_ap[j0 * 128 : (j0 + k) * 128, :].rearrange(
                    "(k p) d -> p k d", k=k
                ),
                g[:].rearrange("p (k d) -> p k d", k=k),
            )


def build(tiles, loop_iters=None, bmax=None, **body_kw):
    """Build the SPMD Bass program (same program for all 8 cores).

    tiles: number of 128-token gather tiles per core (capacity).
    loop_iters: if set, wrap the (idempotent) body in a For_i loop that
    executes it that many times — used only for timing amplification.
    """
    key = ("nc", tiles, loop_iters, bmax, tuple(sorted(body_kw.items())))
    if key in _CACHE:
        return _CACHE[key]
    nc = bacc.Bacc("TRN2", target_bir_lowering=False, debug=False)
    io = declare_io(nc, tiles)
    with tile.TileContext(nc) as tc:
        if loop_iters is None:
            emit_body(nc, tc, io, tiles, bmax=bmax, **body_kw)
        else:
            with tc.For_i(0, loop_iters, 1):
                emit_body(nc, tc, io, tiles, bmax=bmax, **body_kw)
    nc.compile()
    _CACHE[key] = nc
    return nc


def _hash_idx_host(t_flat, p_flat):
    a = (t_flat.astype(np.int64) % SZ) * C_T
    b = (p_flat.astype(np.int64) % SZ) * C_P
    return ((a + b) % SZ).astype(np.int64)


def route(t, tab=None, w_proj=None):
    """Host routing: order tokens by owning core; returns the order and
    per-core counts, plus the padded per-core capacity in 128-token tiles."""
    t = np.asarray(t)
    prev = np.pad(t[:, :-1], ((0, 0), (1, 0)))
    t_flat = np.ascontiguousarray(t, dtype=np.int32).reshape(-1)
    p_flat = np.ascontiguousarray(prev, dtype=np.int32).reshape(-1)
    idx = _hash_idx_host(t_flat, p_flat)
    owner = idx // SLICE
    # sort by full index == sort by (owner, local idx): per-core tokens
    # are then ordered by table row, so gather tile j only touches a
    # prefix of the fold chunks.
    order = np.argsort(idx, kind="stable")
    counts = np.bincount(owner, minlength=N_CORES)
    tiles = max(1, int(-(-counts.max() // 128)))
    return t_flat, p_flat, idx, order, counts, tiles


def make_in_maps(t, tab, w_proj):
    """Host-side marshalling: route tokens, shard table rows, transpose."""
    tab = np.ascontiguousarray(np.asarray(tab), dtype=np.float32)
    w_proj = np.ascontiguousarray(np.asarray(w_proj), dtype=np.float32)
    t_flat, p_flat, idx, order, counts, tiles = route(t)
    cap = tiles * 128

    tabT = np.ascontiguousarray(tab.T)                       # [D, SZ]
    wT = np.ascontiguousarray(w_proj.T).reshape(KC, 128, D)

    in_maps = []
    bmax_per_core = []
    off = 0
    for c in range(N_CORES):
        n = int(counts[c])
        toks = order[off : off + n]
        off += n
        t_sh = np.zeros(cap, np.int32)
        tp_sh = np.zeros(cap, np.int32)
        t_sh[:n] = t_flat[toks]
        tp_sh[:n] = p_flat[toks]
        loc = np.zeros(cap, np.int64)
        loc[:n] = idx[toks] - c * SLICE
        bm = tuple(
            int(loc[j * 128 : min((j + 1) * 128, n)].max() // 128)
            if j * 128 < n else 0
            for j in range(tiles)
        )
        bmax_per_core.append(bm)
        # device layout [128, tiles]: element [p, j] = slot j*128 + p
        t_sh = np.ascontiguousarray(t_sh.reshape(tiles, 128).T)
        tp_sh = np.ascontiguousarray(tp_sh.reshape(tiles, 128).T)
        base = np.full((128, 1), c * SLICE, np.float32)
        tabT_sl = np.ascontiguousarray(
            tabT[:, c * SLICE : (c + 1) * SLICE]
        ).reshape(KC, 128, SLICE)
        in_maps.append(
            {"t_sh": t_sh, "tp_sh": tp_sh, "base": base,
             "tabT": tabT_sl, "w_projT": wT}
        )
    # SPMD: one program for all cores — take the elementwise max over cores
    bmax = tuple(
        max(bmax_per_core[c][j] for c in range(N_CORES)) for j in range(tiles)
    )
    return in_maps, order, counts, tiles, bmax


def kernel(t, tab, w_proj):
    in_maps, order, counts, tiles, bmax = make_in_maps(t, tab, w_proj)
    nc = build(tiles, bmax=bmax)
    res = run_bass_kernel_spmd(nc, in_maps, list(range(N_CORES)))
    out = np.empty((NTOK, D), np.float32)
    off = 0
    for c in range(N_CORES):
        n = int(counts[c])
        out[order[off : off + n]] = res.results[c]["out_sh"][:n]
        off += n
    return out.reshape(B, T, D)



# revision 6
# speedup vs baseline: 1.3367x; 1.3367x over previous
"""Trainium2 Bass kernel for nn_BigramHash: out = tab[hash(t,prev)] @ w_proj.T.

Strategy (v2 — SBUF-resident bf16 table + dma_gather):
  - Table rows are sharded across the 8 cores (384 rows each); each core
    folds its slice on-device: tab2 = tab[rows] @ w_proj.T, computed with
    fp32r matmuls (full-rate on the PE at near-fp32 precision), and keeps
    the result in SBUF as bf16 (0.77 MB) — the gather never touches HBM.
  - The host routes each token to the core owning its hashed row.  Tokens
    are ordered by table-row chunk (so early gather groups depend only on
    early fold chunks) and, within a chunk, round-robined across the row's
    SBUF partition (r % 128) in an AXI-port-interleaved order so the
    SBUF-source gather reads spread across all 16 SBUF ports.
  - Each core recomputes the bigram hash for its tokens on DVE (exact in
    fp32), rebases it into its local slice, and emits int16 indices in the
    dma_gather layout ([16, n/16] blocks replicated across the 8 Q7-core
    partition groups — the replication comes free by replicating the raw
    t/prev inputs host-side).
  - dma_gather (SBUF-source, transpose mode) expands the table rows to
    token order: a handful of instructions replace the 33 indirect DMAs +
    16.9 MB HBM round-trip of v1.  Output tiles [128, 8, gsz] are streamed
    to DRAM as bf16 (half the write traffic of f32).
  - The host de-transposes, upcasts to f32, and scatters rows back to
    token order.

Per-core HBM traffic: ~5.8 MB loads + 8.7 MB output writes (vs ~40 MB in
v1).  Gather traffic rides the SBUF fabric instead of HBM.

Accuracy: the only rounding vs the fp32 reference is fp32r's reduced
multiplier precision in the fold plus one bf16 quantization of tab2 —
elementwise relative error ~2^-9.
"""

import numpy as np

import concourse.bass as bass
import concourse.tile as tile
from concourse import bacc, mybir
from concourse.bass_utils import run_bass_kernel_spmd

N_CORES = 8
B, T = 4, 8192
SZ, D = 3072, 1024
NTOK = B * T                      # 32768
SLICE = SZ // N_CORES             # 384 table rows per core
RC_LOC = SLICE // 128             # 3 fold row-chunks per core
KC = D // 128                     # 8 contraction chunks

C_T = 31337 % SZ                  # 617
C_P = 1000003 % SZ                # 1603

GROUP_TOKENS = 512                # tokens per gather group (multiple of 128)

_CACHE = {}


def declare_io(nc, tiles, fold_dtype="f32r"):
    f32 = mybir.dt.float32
    i32 = mybir.dt.int32
    bf16 = mybir.dt.bfloat16
    mm_dt = {"f32r": mybir.dt.float32r, "f32": f32}[fold_dtype]
    s = tiles * 128 // 16          # columns of the [128, s] token layout
    t_ap = nc.dram_tensor("t_sh", [128, s], i32, kind="ExternalInput").ap()
    tp_ap = nc.dram_tensor("tp_sh", [128, s], i32, kind="ExternalInput").ap()
    base_ap = nc.dram_tensor("base", [128, 1], f32, kind="ExternalInput").ap()
    tabT_ap = nc.dram_tensor(
        "tabT", [KC, 128, SLICE], mm_dt, kind="ExternalInput"
    ).ap()
    wT_ap = nc.dram_tensor("w_projT", [KC, 128, D], mm_dt, kind="ExternalInput").ap()
    out_ap = nc.dram_tensor(
        "out_sh", [128, 8 * tiles * 128], bf16, kind="ExternalOutput"
    ).ap()
    return t_ap, tp_ap, base_ap, tabT_ap, wT_ap, out_ap


def emit_body(nc, tc, io, tiles, groups, fold_dtype="f32r"):
    f32 = mybir.dt.float32
    i32 = mybir.dt.int32
    i16 = mybir.dt.int16
    bf16 = mybir.dt.bfloat16
    t_ap, tp_ap, base_ap, tabT_ap, wT_ap, out_ap = io
    cap = tiles * 128
    s = cap // 16
    with (
        tc.tile_pool(name="weights", bufs=1) as wpool,
        tc.tile_pool(name="psum", bufs=2, space="PSUM") as ppool,
        tc.tile_pool(name="idx", bufs=1) as ipool,
        tc.tile_pool(name="gather", bufs=4) as gpool,
    ):
        # ---- small loads first: hash inputs ----
        t_sb = ipool.tile([128, s], i32)
        nc.scalar.dma_start(t_sb[:], t_ap[:])
        tp_sb = ipool.tile([128, s], i32)
        nc.scalar.dma_start(tp_sb[:], tp_ap[:])
        base_sb = ipool.tile([128, 1], f32)
        nc.scalar.dma_start(base_sb[:], base_ap[:])

        # ---- big loads: w_projT (4 MB) then tabT slice (1.5 MB) ----
        mm_dt = {"f32r": mybir.dt.float32r, "f32": f32}[fold_dtype]
        wT_sb = []
        for kc in range(KC):
            wt = wpool.tile([128, D], mm_dt, tag=f"wT{kc}")
            nc.scalar.dma_start(wt[:], wT_ap[kc])
            wT_sb.append(wt)
        tabT_sb = []
        for kc in range(KC):
            tt = wpool.tile([128, SLICE], mm_dt, tag=f"tabT{kc}")
            nc.scalar.dma_start(tt[:], tabT_ap[kc])
            tabT_sb.append(tt)

        # ---- hash on DVE (exact in fp32; all intermediates < 2^24) ----
        def mod_sz(dst, src):
            m = ipool.tile([128, s], f32, tag="mod_m")
            nc.vector.tensor_scalar(
                m[:], src[:], 1.0 / SZ, None, op0=mybir.AluOpType.mult
            )
            qi = ipool.tile([128, s], i32, tag="mod_qi")
            nc.vector.tensor_copy(qi[:], m[:])
            qf = ipool.tile([128, s], f32, tag="mod_qf")
            nc.vector.tensor_copy(qf[:], qi[:])
            q3 = ipool.tile([128, s], f32, tag="mod_q3")
            nc.vector.tensor_scalar(
                q3[:], qf[:], float(SZ), None, op0=mybir.AluOpType.mult
            )
            nc.vector.tensor_tensor(dst[:], src[:], q3[:], op=mybir.AluOpType.subtract)
            fix = ipool.tile([128, s], f32, tag="mod_fix")
            nc.vector.tensor_scalar(
                fix[:], dst[:], 0.0, float(SZ),
                op0=mybir.AluOpType.is_lt, op1=mybir.AluOpType.mult,
            )
            nc.vector.tensor_tensor(dst[:], dst[:], fix[:], op=mybir.AluOpType.add)
            nc.vector.tensor_scalar(
                fix[:], dst[:], float(SZ), float(-SZ),
                op0=mybir.AluOpType.is_ge, op1=mybir.AluOpType.mult,
            )
            nc.vector.tensor_tensor(dst[:], dst[:], fix[:], op=mybir.AluOpType.add)

        tf = ipool.tile([128, s], f32)
        nc.vector.tensor_copy(tf[:], t_sb[:])
        pf = ipool.tile([128, s], f32)
        nc.vector.tensor_copy(pf[:], tp_sb[:])

        tm = ipool.tile([128, s], f32)
        mod_sz(tm, tf)
        pm = ipool.tile([128, s], f32)
        mod_sz(pm, pf)

        sacc = ipool.tile([128, s], f32)
        nc.vector.tensor_scalar(tm[:], tm[:], float(C_T), None,
                                op0=mybir.AluOpType.mult)
        nc.vector.tensor_scalar(pm[:], pm[:], float(C_P), None,
                                op0=mybir.AluOpType.mult)
        nc.vector.tensor_tensor(sacc[:], tm[:], pm[:], op=mybir.AluOpType.add)
        sf = ipool.tile([128, s], f32)
        mod_sz(sf, sacc)
        # rebase into the local slice and clamp (pad slots and foreign rows
        # are discarded by the host)
        nc.vector.tensor_tensor(sf[:], sf[:],
                                base_sb[:, 0:1].to_broadcast([128, s]),
                                op=mybir.AluOpType.subtract)
        nc.vector.tensor_scalar(sf[:], sf[:], 0.0, float(SLICE - 1),
                                op0=mybir.AluOpType.max, op1=mybir.AluOpType.min)
        idx_sb = ipool.tile([128, s], i16)
        nc.vector.tensor_copy(idx_sb[:], sf[:])

        # ---- fold: tab2 = tab[rows] @ w_proj.T, kept in SBUF as bf16 ----
        tab2_sb = wpool.tile([128, RC_LOC * D], bf16, tag="tab2")
        for rc in range(RC_LOC):
            ps = ppool.tile([128, D], f32)
            for kc in range(KC):
                lhsT = tabT_sb[kc][:, rc * 128 : (rc + 1) * 128]
                nc.tensor.matmul(
                    ps[:, 0:512], lhsT, wT_sb[kc][:, 0:512],
                    start=(kc == 0), stop=(kc == KC - 1),
                )
                nc.tensor.matmul(
                    ps[:, 512:1024], lhsT, wT_sb[kc][:, 512:1024],
                    start=(kc == 0), stop=(kc == KC - 1),
                )
            nc.vector.tensor_copy(tab2_sb[:, rc * D : (rc + 1) * D], ps[:])

        # ---- gather groups: SBUF-source dma_gather + bf16 out stream ----
        for gi, (g0, gsz, span) in enumerate(groups):
            dst = gpool.tile([128, 8, gsz], bf16, tag=f"g{gsz}")
            nc.gpsimd.dma_gather(
                out_ap=dst[:],
                in_ap=tab2_sb[:, : span * D],
                idxs_ap=idx_sb[:, g0 // 16 : (g0 + gsz) // 16],
                num_idxs=gsz,
                num_idxs_reg=gsz,
                elem_size=D,
                transpose=True,
                sbuf_tokens_per_rank=128,
                sbuf_free_dim_per_rank=2 * D,
            )
            nc.sync.dma_start(
                out_ap[:, 8 * g0 : 8 * (g0 + gsz)],
                dst[:].rearrange("p c j -> p (c j)"),
            )


def build(tiles, loop_iters=None, bmax=None, fold_dtype="f32r"):
    """Build the SPMD Bass program (same program for all 8 cores).

    tiles: per-core token capacity in 128-token units.
    bmax: tuple of (g0, gsz, span) gather-group specs (host-derived, maxed
    across cores so the program is identical on every core).
    loop_iters: wrap the idempotent body in a For_i loop (timing only).
    """
    groups = bmax
    key = ("nc", tiles, loop_iters, groups, fold_dtype)
    if key in _CACHE:
        return _CACHE[key]
    nc = bacc.Bacc("TRN2", target_bir_lowering=False, debug=False)
    io = declare_io(nc, tiles, fold_dtype=fold_dtype)
    with tile.TileContext(nc) as tc:
        if loop_iters is None:
            emit_body(nc, tc, io, tiles, groups, fold_dtype=fold_dtype)
        else:
            with tc.For_i(0, loop_iters, 1):
                emit_body(nc, tc, io, tiles, groups, fold_dtype=fold_dtype)
    nc.compile()
    _CACHE[key] = nc
    return nc


def _hash_idx_host(t_flat, p_flat):
    a = (t_flat.astype(np.int64) % SZ) * C_T
    b = (p_flat.astype(np.int64) % SZ) * C_P
    return ((a + b) % SZ).astype(np.int64)


def _port_interleaved_partitions():
    """Permutation of 0..127 such that consecutive entries cycle through
    all 16 SBUF AXI ports (port(p) = 2*((p%32)//4) + (p>=64))."""
    port_of = lambda p: 2 * ((p % 32) // 4) + (1 if p >= 64 else 0)
    by_port = [[] for _ in range(16)]
    for p in range(128):
        by_port[port_of(p)].append(p)
    perm = []
    for k in range(8):
        for q in range(16):
            perm.append(by_port[q][k])
    return np.array(perm, dtype=np.int64)


def route(t):
    """Host routing: owner core + per-core slot order.

    Slot order per core: by fold chunk (so gather groups only depend on a
    prefix of the fold), then round-robin across partitions (row % 128) in
    port-interleaved order to spread SBUF-source gather reads."""
    t = np.asarray(t)
    prev = np.pad(t[:, :-1], ((0, 0), (1, 0)))
    t_flat = np.ascontiguousarray(t, dtype=np.int32).reshape(-1)
    p_flat = np.ascontiguousarray(prev, dtype=np.int32).reshape(-1)
    idx = _hash_idx_host(t_flat, p_flat)
    owner = idx // SLICE
    counts = np.bincount(owner, minlength=N_CORES)
    tiles = max(1, int(-(-counts.max() // 128)))
    cap = tiles * 128

    perm = _port_interleaved_partitions()
    rank_of_part = np.empty(128, np.int64)
    rank_of_part[perm] = np.arange(128)

    loc = idx - owner * SLICE                 # local row in [0, 384)
    chunk = loc // 128
    part = loc % 128
    # emission key per token: (owner, chunk, round k within its partition
    # bucket, port-interleaved partition rank)
    order0 = np.lexsort((part, chunk, owner))  # group by (owner, chunk, part)
    # round index k within each (owner, chunk, part) bucket
    oo, cc, pp = owner[order0], chunk[order0], part[order0]
    bucket_change = np.r_[True, (oo[1:] != oo[:-1]) | (cc[1:] != cc[:-1]) |
                          (pp[1:] != pp[:-1])]
    bucket_id = np.cumsum(bucket_change) - 1
    pos = np.arange(len(order0))
    k_in_bucket = pos - np.maximum.accumulate(np.where(bucket_change, pos, 0))
    key = np.lexsort((rank_of_part[pp], k_in_bucket, cc, oo))
    order = order0[key]                        # final per-core slot order
    return t_flat, p_flat, idx, order, counts, tiles, cap


def make_in_maps(t, tab, w_proj):
    """Host-side marshalling: route tokens, shard table rows, transpose."""
    tab = np.ascontiguousarray(np.asarray(tab), dtype=np.float32)
    w_proj = np.ascontiguousarray(np.asarray(w_proj), dtype=np.float32)
    t_flat, p_flat, idx, order, counts, tiles, cap = route(t)
    s = cap // 16

    tabT = np.ascontiguousarray(tab.T)                       # [D, SZ]
    wT = np.ascontiguousarray(w_proj.T).reshape(KC, 128, D)

    # group layout: fixed sizes, identical across cores
    sizes = []
    left = cap
    while left > 0:
        g = min(GROUP_TOKENS, left)
        sizes.append(g)
        left -= g
    starts = np.cumsum([0] + sizes[:-1]).tolist()

    in_maps = []
    slots_per_core = []
    span_per_core = []
    off = 0
    for c in range(N_CORES):
        n = int(counts[c])
        toks = order[off : off + n]
        off += n
        t_sh = np.zeros(cap, np.int32)
        tp_sh = np.zeros(cap, np.int32)
        t_sh[:n] = t_flat[toks]
        tp_sh[:n] = p_flat[toks]
        # device-visible local row (after clamp) per slot — for group spans
        loc = np.clip(_hash_idx_host(t_sh, tp_sh) - c * SLICE, 0, SLICE - 1)
        spans = [int(loc[g0 : g0 + gs].max() // 128) + 1
                 for g0, gs in zip(starts, sizes)]
        span_per_core.append(spans)
        slots_per_core.append(toks)
        # device layout [128, s] int32: block [16, s] with token i at
        # [i%16, i//16], replicated across the 8 Q7 partition groups
        blk_t = np.ascontiguousarray(t_sh.reshape(s, 16).T)
        blk_p = np.ascontiguousarray(tp_sh.reshape(s, 16).T)
        in_maps.append(
            {
                "t_sh": np.tile(blk_t, (8, 1)),
                "tp_sh": np.tile(blk_p, (8, 1)),
                "base": np.full((128, 1), c * SLICE, np.float32),
                "tabT": np.ascontiguousarray(
                    tabT[:, c * SLICE : (c + 1) * SLICE]
                ).reshape(KC, 128, SLICE),
                "w_projT": wT,
            }
        )
    # SPMD: identical program everywhere -> max span across cores per group
    groups = tuple(
        (int(g0), int(gs), max(span_per_core[c][i] for c in range(N_CORES)))
        for i, (g0, gs) in enumerate(zip(starts, sizes))
    )
    return in_maps, (slots_per_core, counts, sizes, starts), counts, tiles, groups


def kernel(t, tab, w_proj):
    import ml_dtypes

    in_maps, meta, counts, tiles, groups = make_in_maps(t, tab, w_proj)
    slots_per_core, _, sizes, starts = meta
    nc = build(tiles, bmax=groups)
    res = run_bass_kernel_spmd(nc, in_maps, list(range(N_CORES)))
    out = np.empty((NTOK, D), np.float32)
    for c in range(N_CORES):
        n = int(counts[c])
        r = np.asarray(res.results[c]["out_sh"])
        if r.dtype != ml_dtypes.bfloat16:
            r = r.view(ml_dtypes.bfloat16)
        rows = np.empty((tiles * 128, D), np.float32)
        for g0, gs in zip(starts, sizes):
            blk = r[:, 8 * g0 : 8 * (g0 + gs)].reshape(128, 8, gs)
            rows[g0 : g0 + gs] = (
                blk.transpose(2, 1, 0).reshape(gs, D).astype(np.float32)
            )
        out[slots_per_core[c]] = rows[:n]
    return out.reshape(B, T, D)


# revision 31
# speedup vs baseline: 1.7442x; 1.3048x over previous
"""Trainium2 Bass kernel for nn_BigramHash: out = tab[hash(t,prev)] @ w_proj.T.

Strategy (v2 — SBUF-resident bf16 table + dma_gather):
  - Table rows are sharded across the 8 cores (384 rows each); each core
    folds its slice on-device: tab2 = tab[rows] @ w_proj.T, computed with
    fp32r matmuls (full-rate on the PE at near-fp32 precision), and keeps
    the result in SBUF as bf16 (0.77 MB) — the gather never touches HBM.
  - The host routes each token to the core owning its hashed row.  Tokens
    are ordered by table-row chunk (so early gather groups depend only on
    early fold chunks) and, within a chunk, round-robined across the row's
    SBUF partition (r % 128) in an AXI-port-interleaved order so the
    SBUF-source gather reads spread across all 16 SBUF ports.
  - Each core recomputes the bigram hash for its tokens on DVE (exact in
    fp32), rebases it into its local slice, and emits int16 indices in the
    dma_gather layout ([16, n/16] blocks replicated across the 8 Q7-core
    partition groups — the replication comes free by replicating the raw
    t/prev inputs host-side).
  - dma_gather (SBUF-source, transpose mode) expands the table rows to
    token order: a handful of instructions replace the 33 indirect DMAs +
    16.9 MB HBM round-trip of v1.  Output tiles [128, 8, gsz] are streamed
    to DRAM as bf16 (half the write traffic of f32).
  - The host de-transposes, upcasts to f32, and scatters rows back to
    token order.

Per-core HBM traffic: ~5.8 MB loads + 8.7 MB output writes (vs ~40 MB in
v1).  Gather traffic rides the SBUF fabric instead of HBM.

Accuracy: the only rounding vs the fp32 reference is fp32r's reduced
multiplier precision in the fold plus one bf16 quantization of tab2 —
elementwise relative error ~2^-9.
"""

import numpy as np

import concourse.bass as bass
import concourse.tile as tile
from concourse import bacc, mybir
from concourse.bass_utils import run_bass_kernel_spmd

N_CORES = 8
B, T = 4, 8192
SZ, D = 3072, 1024
NTOK = B * T                      # 32768
SLICE = SZ // N_CORES             # 384 table rows per core
RC_LOC = SLICE // 128             # 3 fold row-chunks per core
KC = D // 128                     # 8 contraction chunks

C_T = 31337 % SZ                  # 617
C_P = 1000003 % SZ                # 1603

GROUP_TOKENS = 512                # tokens per gather group (multiple of 128)
GATHER_SRC = "dram"               # "dram" (non-transpose) | "sbuf" (transpose)
FOLD_DTYPE = "bf16"               # "f32r" | "f32" | "bf16"

_CACHE = {}


def declare_io(nc, tiles, fold_dtype="f32r"):
    f32 = mybir.dt.float32
    i32 = mybir.dt.int32
    bf16 = mybir.dt.bfloat16
    mm_dt = {"f32r": mybir.dt.float32r, "f32": f32, "bf16": bf16}[fold_dtype]
    s = tiles * 128 // 16          # columns of the [128, s] token layout
    t_ap = nc.dram_tensor("t_sh", [128, s], i32, kind="ExternalInput").ap()
    tp_ap = nc.dram_tensor("tp_sh", [128, s], i32, kind="ExternalInput").ap()
    base_ap = nc.dram_tensor("base", [128, 1], f32, kind="ExternalInput").ap()
    tabT_ap = nc.dram_tensor(
        "tabT", [KC, 128, SLICE], mm_dt, kind="ExternalInput"
    ).ap()
    wT_ap = nc.dram_tensor("w_projT", [KC, 128, D], mm_dt, kind="ExternalInput").ap()
    out_ap = nc.dram_tensor(
        "out_sh", [128, 8 * tiles * 128], bf16, kind="ExternalOutput"
    ).ap()
    tab2_ap = nc.dram_tensor("tab2", [SLICE, D], bf16).ap()
    return t_ap, tp_ap, base_ap, tabT_ap, wT_ap, out_ap, tab2_ap


def emit_body(nc, tc, io, tiles, groups, fold_dtype=FOLD_DTYPE,
              do_hash=True, do_fold=True, do_gather=True, do_out=True,
              gather_src=GATHER_SRC, single_packet=True):
    f32 = mybir.dt.float32
    i32 = mybir.dt.int32
    i16 = mybir.dt.int16
    bf16 = mybir.dt.bfloat16
    t_ap, tp_ap, base_ap, tabT_ap, wT_ap, out_ap, tab2_ap = io
    cap = tiles * 128
    s = cap // 16
    with (
        tc.tile_pool(name="weights", bufs=1) as wpool,
        tc.tile_pool(name="psum", bufs=2, space="PSUM") as ppool,
        tc.tile_pool(name="idx", bufs=1) as ipool,
        tc.tile_pool(name="gather", bufs=4) as gpool,
    ):
        # ---- small loads first: hash inputs ----
        t_sb = ipool.tile([128, s], i32)
        nc.scalar.dma_start(t_sb[:], t_ap[:])
        tp_sb = ipool.tile([128, s], i32)
        nc.scalar.dma_start(tp_sb[:], tp_ap[:])
        base_sb = ipool.tile([128, 1], f32)
        nc.scalar.dma_start(base_sb[:], base_ap[:])

        # ---- big loads: w_projT first (gates every fold chunk), then the
        # tabT slice in per-fold-chunk column blocks so fold chunk 0 can
        # start as soon as its third of tabT lands ----
        mm_dt = {"f32r": mybir.dt.float32r, "f32": f32, "bf16": bf16}[fold_dtype]
        wT_sb = []
        for kc in range(KC):
            wt = wpool.tile([128, D], mm_dt, tag=f"wT{kc}")
            nc.scalar.dma_start(wt[:], wT_ap[kc])
            wT_sb.append(wt)
        tabT_sb = []
        for kc in range(KC):
            tt = wpool.tile([128, SLICE], mm_dt, tag=f"tabT{kc}")
            tabT_sb.append(tt)
        for rc in range(RC_LOC):
            for kc in range(KC):
                nc.scalar.dma_start(
                    tabT_sb[kc][:, rc * 128 : (rc + 1) * 128],
                    tabT_ap[kc][:, rc * 128 : (rc + 1) * 128],
                )

        # ---- hash on DVE (exact in fp32; all intermediates < 2^24) ----
        idx_sb = ipool.tile([128, s], i16)
        tpf = ipool.tile([128, 2 * s], f32)
        tpm = ipool.tile([128, 2 * s], f32)
        m_t = ipool.tile([128, 2 * s], f32, tag="mod_m")
        qi_t = ipool.tile([128, 2 * s], i32, tag="mod_qi")
        qf_t = ipool.tile([128, 2 * s], f32, tag="mod_qf")
        fx_t = ipool.tile([128, 2 * s], f32, tag="mod_fix")
        sacc = ipool.tile([128, s], f32)
        sf = ipool.tile([128, s], f32)

        def mod_sz(dst, src, lo, w):
            m, qi, qf, fix = (m_t[:, lo : lo + w], qi_t[:, lo : lo + w],
                              qf_t[:, lo : lo + w], fx_t[:, lo : lo + w])
            nc.vector.tensor_scalar(m, src, 1.0 / SZ, None,
                                    op0=mybir.AluOpType.mult)
            nc.vector.tensor_copy(qi, m)
            nc.vector.tensor_copy(qf, qi)
            nc.vector.tensor_scalar(qf, qf, float(SZ), None,
                                    op0=mybir.AluOpType.mult)
            nc.vector.tensor_tensor(dst, src, qf, op=mybir.AluOpType.subtract)
            nc.vector.tensor_scalar(fix, dst, 0.0, float(SZ),
                                    op0=mybir.AluOpType.is_lt,
                                    op1=mybir.AluOpType.mult)
            nc.vector.tensor_tensor(dst, dst, fix, op=mybir.AluOpType.add)
            nc.vector.tensor_scalar(fix, dst, float(SZ), float(-SZ),
                                    op0=mybir.AluOpType.is_ge,
                                    op1=mybir.AluOpType.mult)
            nc.vector.tensor_tensor(dst, dst, fix, op=mybir.AluOpType.add)

        def hash_block(c0, c1):
            """Compute idx_sb[:, c0:c1] (tokens 16*c0 .. 16*c1)."""
            w = c1 - c0
            t2, p2 = 2 * c0, 2 * c0 + w
            nc.vector.tensor_copy(tpf[:, t2 : t2 + w], t_sb[:, c0:c1])
            nc.vector.tensor_copy(tpf[:, p2 : p2 + w], tp_sb[:, c0:c1])
            mod_sz(tpm[:, t2 : t2 + 2 * w], tpf[:, t2 : t2 + 2 * w], t2, 2 * w)
            nc.vector.tensor_scalar(tpm[:, t2 : t2 + w], tpm[:, t2 : t2 + w],
                                    float(C_T), None, op0=mybir.AluOpType.mult)
            nc.vector.tensor_scalar(tpm[:, p2 : p2 + w], tpm[:, p2 : p2 + w],
                                    float(C_P), None, op0=mybir.AluOpType.mult)
            nc.vector.tensor_tensor(sacc[:, c0:c1], tpm[:, t2 : t2 + w],
                                    tpm[:, p2 : p2 + w], op=mybir.AluOpType.add)
            mod_sz(sf[:, c0:c1], sacc[:, c0:c1], c0, w)
            # rebase into the local slice and clamp (pad slots and foreign
            # rows are discarded by the host)
            nc.vector.tensor_tensor(sf[:, c0:c1], sf[:, c0:c1],
                                    base_sb[:, 0:1].to_broadcast([128, w]),
                                    op=mybir.AluOpType.subtract)
            nc.vector.tensor_scalar(sf[:, c0:c1], sf[:, c0:c1], 0.0,
                                    float(SLICE - 1),
                                    op0=mybir.AluOpType.max,
                                    op1=mybir.AluOpType.min)
            nc.vector.tensor_copy(idx_sb[:, c0:c1], sf[:, c0:c1])

        if do_hash:
            # first block just covers gather group 0 so its indices are
            # ready while the weights are still loading
            first = groups[0][1] // 16
            hash_block(0, first)
            if first < s:
                hash_block(first, s)
        else:
            nc.vector.memset(idx_sb[:], 0)

        # ---- fold: tab2 = tab[rows] @ w_proj.T, kept in SBUF as bf16 ----
        tab2_sb = wpool.tile([128, RC_LOC * D], bf16, tag="tab2")
        if not do_fold:
            nc.vector.memset(tab2_sb[:], 0)
        for rc in range(RC_LOC if do_fold else 0):
            ps = ppool.tile([128, D], f32)
            for kc in range(KC):
                lhsT = tabT_sb[kc][:, rc * 128 : (rc + 1) * 128]
                nc.tensor.matmul(
                    ps[:, 0:512], lhsT, wT_sb[kc][:, 0:512],
                    start=(kc == 0), stop=(kc == KC - 1),
                )
                nc.tensor.matmul(
                    ps[:, 512:1024], lhsT, wT_sb[kc][:, 512:1024],
                    start=(kc == 0), stop=(kc == KC - 1),
                )
            nc.vector.tensor_copy(tab2_sb[:, rc * D : (rc + 1) * D], ps[:])
            if gather_src == "dram":
                nc.sync.dma_start(
                    tab2_ap[rc * 128 : (rc + 1) * 128, :],
                    tab2_sb[:, rc * D : (rc + 1) * D],
                )

        # ---- gather groups: SBUF-source dma_gather + bf16 out stream ----
        for gi, (g0, gsz, span) in enumerate(groups):
            if gather_src == "sbuf":
                dst = gpool.tile([128, 8, gsz], bf16, tag=f"g{gsz}")
            else:
                dst = gpool.tile([128, gsz // 128, D], bf16, tag=f"g{gsz}")
            if do_gather:
                if gather_src == "sbuf":
                    nc.gpsimd.dma_gather(
                        out_ap=dst[:],
                        in_ap=tab2_sb[:, : span * D],
                        idxs_ap=idx_sb[:, g0 // 16 : (g0 + gsz) // 16],
                        num_idxs=gsz,
                        num_idxs_reg=gsz,
                        elem_size=D,
                        transpose=True,
                        sbuf_tokens_per_rank=128,
                        sbuf_free_dim_per_rank=2 * D,
                        single_packet=single_packet,
                    )
                else:
                    nc.gpsimd.dma_gather(
                        out_ap=dst[:],
                        in_ap=tab2_ap[0 : span * 128, :],
                        idxs_ap=idx_sb[:, g0 // 16 : (g0 + gsz) // 16],
                        num_idxs=gsz,
                        num_idxs_reg=gsz,
                        elem_size=D,
                        single_packet=single_packet,
                    )
            elif not do_out:
                continue
            if do_out:
                if gather_src == "sbuf":
                    src = dst[:].rearrange("p c j -> p (c j)")
                else:
                    src = dst[:].rearrange("p j d -> p (j d)")
                out_eng = nc.sync if gi % 2 == 0 else nc.scalar
                out_eng.dma_start(out_ap[:, 8 * g0 : 8 * (g0 + gsz)], src)


def build(tiles, loop_iters=None, bmax=None, fold_dtype=FOLD_DTYPE, **body_kw):
    """Build the SPMD Bass program (same program for all 8 cores).

    tiles: per-core token capacity in 128-token units.
    bmax: tuple of (g0, gsz, span) gather-group specs (host-derived, maxed
    across cores so the program is identical on every core).
    loop_iters: wrap the idempotent body in a For_i loop (timing only).
    """
    groups = bmax
    key = ("nc", tiles, loop_iters, groups, fold_dtype,
           tuple(sorted(body_kw.items())))
    if key in _CACHE:
        return _CACHE[key]
    nc = bacc.Bacc("TRN2", target_bir_lowering=False, debug=False)
    io = declare_io(nc, tiles, fold_dtype=fold_dtype)
    with tile.TileContext(nc) as tc:
        if loop_iters is None:
            emit_body(nc, tc, io, tiles, groups, fold_dtype=fold_dtype, **body_kw)
        else:
            with tc.For_i(0, loop_iters, 1):
                emit_body(nc, tc, io, tiles, groups, fold_dtype=fold_dtype,
                          **body_kw)
    nc.compile()
    _CACHE[key] = nc
    return nc


def _hash_idx_host(t_flat, p_flat):
    a = (t_flat.astype(np.int64) % SZ) * C_T
    b = (p_flat.astype(np.int64) % SZ) * C_P
    return ((a + b) % SZ).astype(np.int64)


def _port_interleaved_partitions():
    """Permutation of 0..127 such that consecutive entries cycle through
    all 16 SBUF AXI ports (port(p) = 2*((p%32)//4) + (p>=64))."""
    port_of = lambda p: 2 * ((p % 32) // 4) + (1 if p >= 64 else 0)
    by_port = [[] for _ in range(16)]
    for p in range(128):
        by_port[port_of(p)].append(p)
    perm = []
    for k in range(8):
        for q in range(16):
            perm.append(by_port[q][k])
    return np.array(perm, dtype=np.int64)


def route(t):
    """Host routing: owner core + per-core slot order.

    Slot order per core: by fold chunk (so gather groups only depend on a
    prefix of the fold), then round-robin across partitions (row % 128) in
    port-interleaved order to spread SBUF-source gather reads."""
    t = np.asarray(t)
    prev = np.pad(t[:, :-1], ((0, 0), (1, 0)))
    t_flat = np.ascontiguousarray(t, dtype=np.int32).reshape(-1)
    p_flat = np.ascontiguousarray(prev, dtype=np.int32).reshape(-1)
    idx = _hash_idx_host(t_flat, p_flat)
    owner = idx // SLICE
    counts = np.bincount(owner, minlength=N_CORES)
    tiles = max(1, int(-(-counts.max() // 128)))
    cap = tiles * 128

    perm = _port_interleaved_partitions()
    rank_of_part = np.empty(128, np.int64)
    rank_of_part[perm] = np.arange(128)

    loc = idx - owner * SLICE                 # local row in [0, 384)
    chunk = loc // 128
    part = loc % 128
    # emission key per token: (owner, chunk, round k within its partition
    # bucket, port-interleaved partition rank)
    order0 = np.lexsort((part, chunk, owner))  # group by (owner, chunk, part)
    # round index k within each (owner, chunk, part) bucket
    oo, cc, pp = owner[order0], chunk[order0], part[order0]
    bucket_change = np.r_[True, (oo[1:] != oo[:-1]) | (cc[1:] != cc[:-1]) |
                          (pp[1:] != pp[:-1])]
    bucket_id = np.cumsum(bucket_change) - 1
    pos = np.arange(len(order0))
    k_in_bucket = pos - np.maximum.accumulate(np.where(bucket_change, pos, 0))
    key = np.lexsort((rank_of_part[pp], k_in_bucket, cc, oo))
    order = order0[key]                        # final per-core slot order
    return t_flat, p_flat, idx, order, counts, tiles, cap


def make_in_maps(t, tab, w_proj, fold_dtype=FOLD_DTYPE):
    """Host-side marshalling: route tokens, shard table rows, transpose."""
    tab = np.ascontiguousarray(np.asarray(tab), dtype=np.float32)
    w_proj = np.ascontiguousarray(np.asarray(w_proj), dtype=np.float32)
    t_flat, p_flat, idx, order, counts, tiles, cap = route(t)
    s = cap // 16

    tabT = np.ascontiguousarray(tab.T)                       # [D, SZ]
    wT = np.ascontiguousarray(w_proj.T).reshape(KC, 128, D)
    if fold_dtype == "bf16":
        import ml_dtypes
        tabT = tabT.astype(ml_dtypes.bfloat16)
        wT = wT.astype(ml_dtypes.bfloat16)

    # group layout: fixed sizes, identical across cores
    sizes = []
    left = cap
    while left > 0:
        g = min(GROUP_TOKENS, left)
        sizes.append(g)
        left -= g
    starts = np.cumsum([0] + sizes[:-1]).tolist()

    in_maps = []
    slots_per_core = []
    span_per_core = []
    off = 0
    for c in range(N_CORES):
        n = int(counts[c])
        toks = order[off : off + n]
        off += n
        t_sh = np.zeros(cap, np.int32)
        tp_sh = np.zeros(cap, np.int32)
        t_sh[:n] = t_flat[toks]
        tp_sh[:n] = p_flat[toks]
        # device-visible local row (after clamp) per slot — for group spans
        loc = np.clip(_hash_idx_host(t_sh, tp_sh) - c * SLICE, 0, SLICE - 1)
        spans = [int(loc[g0 : g0 + gs].max() // 128) + 1
                 for g0, gs in zip(starts, sizes)]
        span_per_core.append(spans)
        slots_per_core.append(toks)
        # device layout [128, s] int32: block [16, s] with token i at
        # [i%16, i//16], replicated across the 8 Q7 partition groups
        blk_t = np.ascontiguousarray(t_sh.reshape(s, 16).T)
        blk_p = np.ascontiguousarray(tp_sh.reshape(s, 16).T)
        in_maps.append(
            {
                "t_sh": np.tile(blk_t, (8, 1)),
                "tp_sh": np.tile(blk_p, (8, 1)),
                "base": np.full((128, 1), c * SLICE, np.float32),
                "tabT": np.ascontiguousarray(
                    tabT[:, c * SLICE : (c + 1) * SLICE]
                ).reshape(KC, 128, SLICE),
                "w_projT": wT,
            }
        )
    # SPMD: identical program everywhere -> max span across cores per group
    groups = tuple(
        (int(g0), int(gs), max(span_per_core[c][i] for c in range(N_CORES)))
        for i, (g0, gs) in enumerate(zip(starts, sizes))
    )
    return in_maps, (slots_per_core, counts, sizes, starts), counts, tiles, groups


def kernel(t, tab, w_proj):
    import ml_dtypes

    in_maps, meta, counts, tiles, groups = make_in_maps(t, tab, w_proj)
    slots_per_core, _, sizes, starts = meta
    nc = build(tiles, bmax=groups)
    res = run_bass_kernel_spmd(nc, in_maps, list(range(N_CORES)))
    out = np.empty((NTOK, D), np.float32)
    for c in range(N_CORES):
        n = int(counts[c])
        r = np.asarray(res.results[c]["out_sh"])
        if r.dtype != ml_dtypes.bfloat16:
            r = r.view(ml_dtypes.bfloat16)
        rows = np.empty((tiles * 128, D), np.float32)
        for g0, gs in zip(starts, sizes):
            blk = r[:, 8 * g0 : 8 * (g0 + gs)]
            if GATHER_SRC == "sbuf":
                # blk[p, c*gs + i] = token (g0+i) element c*128+p
                rows[g0 : g0 + gs] = (
                    blk.reshape(128, 8, gs).transpose(2, 1, 0)
                    .reshape(gs, D).astype(np.float32)
                )
            else:
                # blk[p, j*D + d] = token (g0 + j*128 + p) element d
                rows[g0 : g0 + gs] = (
                    blk.reshape(128, gs // 128, D).transpose(1, 0, 2)
                    .reshape(gs, D).astype(np.float32)
                )
        out[slots_per_core[c]] = rows[:n]
    return out.reshape(B, T, D)
